# revision 11
# baseline (speedup 1.0000x reference)
"""ClusterGCN layer on 8 TRN2 NeuronCores.

Math per cluster c (only intra-cluster edges matter):
    Y_c = B_c @ X_c @ W + b,   B_c[d, s] = dis[d] * At_c[s, d] * dis[s]
    At_c[s, d] = #edges(s->d in c) + [s == d],  dis = rsqrt(deg)
Clusters with no intra edge pass X through (patched on host).

Device formulation (At-first, contraction always on the partition dim):
  step1: T^T[k, d] = sum_s Xs[s, k] * At[s, d]   per cluster
         lhsT = Xs chunks (stationary, [s:128, k:128]),
         rhs  = At s-chunk rows ([s:128, d:D], fp8 counts — integers
         <= 16 are exact in e4m3).  Xs = X * dis[s] (folded on host).
  step2: Y^T[f, d] = sum_k W[k, f] * T^T[k, d]
         lhsT = W chunks (stationary, only 4 distinct), rhs = T^T from
         SBUF.  Host applies dis[d] and + b after the gather.
The d (destination-node) axis is trimmed to the true max cluster size D
(<= cap); the s axis stays padded to cap for 128-row contraction chunks.

A junk-data warm-up matmul burst runs at kernel start with no data
dependencies: it spans the initial DMA latency so the PE HAM clock
gate reaches 8/8 (2.4 GHz) before the first real matmul arrives.

DMA is spread over three descriptor queues so the head isn't serialized
on one ring: X tiles + Y stores on Sync (HWDGE), cluster 0's At halves
on Scalar (HWDGE, ahead of its ACT copies), W + remaining At groups on
GpSimd (SWDGE, otherwise idle).  PSUM->SBUF moves split across Vector
and Scalar (GpSimd has no PSUM port on TRN2).
"""

import numpy as np

N_CORES = 8
N_CLUSTERS = 100
P = 128
WARM_MMS = 32

_prog_cache: dict = {}


def _build_program(cpc: int, cap: int, D: int, in_c: int, f_out: int,
                   fp8_path: bool):
    """Build + compile the per-core Bass program.

    cpc: clusters per core; cap: padded cluster size (multiple of 128);
    D: true d-axis extent (<= min(cap, 512)).
    fp8_path: adjacency ships as raw fp8e4m3 counts (exact); the fp16
    fallback ships pre-scaled B^T blocks (counts > 16 only).
    """
    import concourse.mybir as mybir
    import concourse.tile as tile
    from concourse import bacc

    key = (cpc, cap, D, in_c, f_out, fp8_path)
    if key in _prog_cache:
        return _prog_cache[key]

    kc = in_c // P           # k chunks (step-1 stationary cols / step-2 contraction)
    sch = cap // P           # s chunks per cluster (step-1 contraction)
    fc = f_out // P          # f chunks (step-2 output partitions)
    f16 = mybir.dt.float16
    f32 = mybir.dt.float32
    a_dt = mybir.dt.float8e4 if fp8_path else f16

    XG = 4                   # clusters per X load group
    AG = 3                   # clusters per At load group

    nc = bacc.Bacc("TRN2", target_bir_lowering=False, debug=False,
                   num_devices=N_CORES)

    XS = nc.dram_tensor("XS", [P, cpc, sch, in_c], f16, kind="ExternalInput")
    WT = nc.dram_tensor("WT", [P, kc, fc, P], f16, kind="ExternalInput")
    AT = nc.dram_tensor("AT", [cpc, P, sch, D], a_dt, kind="ExternalInput")
    YT = nc.dram_tensor("YT", [cpc, f_out, D], f16, kind="ExternalOutput")

    with tile.TileContext(nc) as tc:
        with (
            tc.tile_pool(name="w", bufs=1) as w_pool,
            tc.tile_pool(name="jk", bufs=1) as jk_pool,
            tc.tile_pool(name="xt", bufs=4) as xt_pool,
            tc.tile_pool(name="at", bufs=4) as at_pool,
            tc.tile_pool(name="tsb", bufs=4) as tsb_pool,
            tc.tile_pool(name="out", bufs=4) as out_pool,
            tc.tile_pool(name="psw", bufs=1, space="PSUM") as psw_pool,
            tc.tile_pool(name="ps1", bufs=3, space="PSUM") as ps1_pool,
            tc.tile_pool(name="ps2", bufs=4, space="PSUM") as ps2_pool,
        ):
            # warm-up burst: junk matmuls with no input deps keep the PE
            # busy through the initial DMA latency and un-throttle HAM
            junk = jk_pool.tile([P, P], f16)
            jout = jk_pool.tile([P, 8], f16)
            nc.gpsimd.memset(junk[:], 0.0)
            psw = psw_pool.tile([P, P], f32)
            for i in range(WARM_MMS):
                nc.tensor.matmul(
                    psw[:], lhsT=junk[:], rhs=junk[:],
                    start=(i == 0), stop=(i == WARM_MMS - 1),
                )
            nc.vector.tensor_copy(jout[:], psw[:, :8])

            # W on the otherwise-idle GpSimd queue; needed only by step 2
            wt = w_pool.tile([P, kc, fc, P], f16)
            nc.gpsimd.dma_start(wt[:], WT[:])

            # small first groups so compute starts early
            def groups(g):
                sizes, c0 = [], 0
                first = True
                while c0 < cpc:
                    g_ = 1 if first else min(g, cpc - c0)
                    sizes.append((c0, g_))
                    c0 += g_
                    first = False
                return sizes

            xg_of = {}
            for c0, g in groups(XG):
                for c in range(c0, c0 + g):
                    xg_of[c] = (c0, g)
            ag_of = {}
            for c0, g in groups(AG):
                for c in range(c0, c0 + g):
                    ag_of[c] = (c0, g)

            xt = at = None
            for c in range(cpc):
                a0, ag = ag_of[c]
                if c == a0:
                    at = at_pool.tile([P, AG, sch, D], a_dt)
                    if c == 0:
                        # cluster 0 on the fast Sync ring, ahead of X
                        nc.sync.dma_start(at[:, 0], AT[0])
                    else:
                        nc.gpsimd.dma_start(
                            at[:, :ag],
                            AT[a0:a0 + ag].rearrange("c p s d -> p c s d"),
                        )
                c0, g = xg_of[c]
                if c == c0:
                    xt = xt_pool.tile([P, XG, sch, in_c], f16)
                    nc.sync.dma_start(xt[:, :g], XS[:, c0:c0 + g])
                xi = c - c0
                ci = c - a0

                # step 1: T^T = sum_s Xs-chunk^T x At rows, k on partitions
                tsb = tsb_pool.tile([P, kc, D], f16)
                for k in range(kc):
                    ps = ps1_pool.tile([P, D], f32)
                    for s in range(sch):
                        nc.tensor.matmul(
                            ps[:],
                            lhsT=xt[:, xi, s, k * P:(k + 1) * P],
                            rhs=at[:, ci, s, :],
                            start=(s == 0),
                            stop=(s == sch - 1),
                        )
                    if k == 0:
                        nc.vector.tensor_copy(tsb[:, k, :], ps[:])
                    else:
                        nc.scalar.copy(tsb[:, k, :], ps[:])

                # step 2: Y^T = sum_k W-chunk^T x T^T, f on partitions
                ot = out_pool.tile([P, fc, D], f16)
                for f in range(fc):
                    ps = ps2_pool.tile([P, D], f32)
                    for k in range(kc):
                        nc.tensor.matmul(
                            ps[:],
                            lhsT=wt[:, k, f, :],
                            rhs=tsb[:, k, :],
                            start=(k == 0),
                            stop=(k == kc - 1),
                        )
                    if f == 0:
                        nc.vector.tensor_copy(ot[:, f, :], ps[:])
                    else:
                        nc.scalar.copy(ot[:, f, :], ps[:])
                YTc = YT[c].rearrange("(f p) d -> p f d", p=P)
                if c == cpc - 1:
                    # split the final store so f0 ships while f1 drains
                    for f in range(fc):
                        nc.sync.dma_start(YTc[:, f:f + 1], ot[:, f:f + 1])
                else:
                    nc.sync.dma_start(YTc, ot[:])

    nc.compile()
    _prog_cache[key] = nc
    return nc


def _host_prep(X, W, b, assign, full_ei):
    """Shard + preprocess. Returns (in_maps, fp8_path, gather info)."""
    n, in_c = X.shape
    f_out = W.shape[1]
    src = full_ei[0].astype(np.int64)
    dst = full_ei[1].astype(np.int64)
    a_s = assign[src]
    intra = a_s == assign[dst]
    es, ed = src[intra], dst[intra]

    deg = np.ones(n, np.float32)
    np.add.at(deg, ed, np.float32(1))
    dis = (1.0 / np.sqrt(deg)).astype(np.float32)

    has_edge = np.zeros(N_CLUSTERS, bool)
    has_edge[np.unique(a_s[intra])] = True

    sizes = np.bincount(assign, minlength=N_CLUSTERS)
    cpc = -(-N_CLUSTERS // N_CORES)            # clusters per core
    cap = max(512, int(-(-sizes.max() // P)) * P)  # padded cluster size
    D = int(sizes.max()) if sizes.max() <= 512 else cap  # true d extent
    kc = in_c // P
    fc = f_out // P
    sch = cap // P

    starts = np.zeros(N_CLUSTERS + 1, np.int64)
    starts[1:] = np.cumsum(sizes)
    order = np.argsort(assign, kind="stable")
    pos = np.empty(n, np.int64)
    pos[order] = np.arange(n) - starts[assign[order]]

    ctot = cpc * N_CORES
    # At blocks: At[c][s, d] = #edges(s->d) + [s==d]
    At = np.zeros((ctot, cap, cap), np.uint16)
    np.add.at(At, (assign[es], pos[es], pos[ed]), 1)
    At[assign, pos, pos] += 1
    fp8_path = int(At.max()) <= 16    # integers <= 16 are exact in e4m3

    Xp = np.zeros((ctot, cap, in_c), np.float32)
    if fp8_path:
        # fold dis[s] into the node features; host applies dis[d] at the end
        Xp[assign, pos] = X * dis[:, None]
        import concourse.mybir as mybir
        At_send = At.astype(mybir.dt.np(mybir.dt.float8e4))
    else:
        # rare fallback: fully pre-scaled B^T blocks in fp16
        Xp[assign, pos] = X
        DISp = np.zeros((ctot, cap), np.float32)
        DISp[assign, pos] = dis
        At_send = (At.astype(np.float32)
                   * DISp[:, :, None] * DISp[:, None, :]).astype(np.float16)
    # [c, s, d] -> [c, p, so, d]: partition p holds rows s = so*P + p
    At_send = np.ascontiguousarray(
        At_send.reshape(-1, sch, P, cap).transpose(0, 2, 1, 3)[..., :D])
    # X: [c, s, k] -> [p, c, so, k]
    XS_all = np.ascontiguousarray(
        Xp.reshape(ctot, sch, P, in_c).transpose(2, 0, 1, 3)
    ).astype(np.float16)
    WT_send = np.ascontiguousarray(
        W.astype(np.float32).reshape(kc, P, fc, P).transpose(1, 0, 2, 3)
    ).astype(np.float16)

    in_maps = []
    for i in range(N_CORES):
        in_maps.append({
            "XS": np.ascontiguousarray(XS_all[:, i * cpc:(i + 1) * cpc]),
            "WT": WT_send,
            "AT": At_send[i * cpc:(i + 1) * cpc],
        })
    return in_maps, fp8_path, (cpc, cap, D, has_edge, pos, dis)


def _run(inputs, trace=False, tmpdir=None):
    from concourse.bass_utils import run_bass_kernel_spmd

    X = np.asarray(inputs["X"], np.float32)
    W = np.asarray(inputs["W"], np.float32)
    b = np.asarray(inputs["b"], np.float32)
    assign = np.asarray(inputs["assign"])
    full_ei = np.asarray(inputs["full_ei"])

    n, in_c = X.shape
    f_out = W.shape[1]
    in_maps, fp8_path, (cpc, cap, D, has_edge, pos, dis) = _host_prep(
        X, W, b, assign, full_ei)
    nc = _build_program(cpc, cap, D, in_c, f_out, fp8_path)

    res = run_bass_kernel_spmd(
        nc, in_maps, core_ids=list(range(N_CORES)),
        trace=trace, tmpdir=tmpdir,
    )
    # YT: [core][cpc, f_out, D]; row n lives at [core, lc, :, pos]
    YTdev = np.stack([res.results[i]["YT"] for i in range(N_CORES)])
    if YTdev.dtype != np.float32:
        YTdev = YTdev.astype(np.float32)

    c = assign.astype(np.int64)
    core = c // cpc
    lc = c % cpc
    Y = YTdev[core, lc, :, pos]
    if fp8_path:
        Y *= dis[:, None]
    Y += b[None, :].astype(np.float32)
    miss = ~has_edge[c]
    if miss.any():
        Y[miss] = X[miss]
    return Y, res


def kernel(**inputs) -> np.ndarray:
    Y, _ = _run(inputs)
    return Y


# revision 15
# speedup vs baseline: 1.1324x; 1.1324x over previous
"""ClusterGCN layer on 8 TRN2 NeuronCores.

Math per cluster c (only intra-cluster edges matter):
    Y_c = B_c @ X_c @ W + b,   B_c[d, s] = dis[d] * At_c[s, d] * dis[s]
    At_c[s, d] = #edges(s->d in c) + [s == d],  dis = rsqrt(deg)
Clusters with no intra edge pass X through (patched on host).

Device formulation (At-first, contraction always on the partition dim):
  step1: T^T[k, d] = sum_s Xs[s, k] * At[s, d]   per cluster
         lhsT = Xs chunks (stationary, [s:128, k:128]),
         rhs  = At s-chunk rows ([s:128, d:D], fp8 counts — integers
         <= 16 are exact in e4m3).  Xs = X * dis[s] (folded on host).
  step2: Y^T[f, d] = sum_k W[k, f] * T^T[k, d]
         lhsT = W chunks (stationary, only 4 distinct), rhs = T^T from
         SBUF.  Host applies dis[d] and + b after the gather.
The d (destination-node) axis is trimmed to the true max cluster size D
(<= cap); the s axis stays padded to cap for 128-row contraction chunks.

A junk-data warm-up matmul burst runs at kernel start with no data
dependencies: it spans the initial DMA latency so the PE HAM clock
gate reaches 8/8 (2.4 GHz) before the first real matmul arrives.

DMA is spread over three descriptor queues so the head isn't serialized
on one ring: X tiles + Y stores on Sync (HWDGE), cluster 0's At halves
on Scalar (HWDGE, ahead of its ACT copies), W + remaining At groups on
GpSimd (SWDGE, otherwise idle).  PSUM->SBUF moves split across Vector
and Scalar (GpSimd has no PSUM port on TRN2).
"""

import numpy as np

N_CORES = 8
N_CLUSTERS = 100
P = 128
WARM_MMS = 30

_prog_cache: dict = {}


def _build_program(cpc: int, cap: int, D: int, in_c: int, f_out: int,
                   fp8_path: bool):
    """Build + compile the per-core Bass program.

    cpc: clusters per core; cap: padded cluster size (multiple of 128);
    D: true d-axis extent (<= min(cap, 512)).
    fp8_path: adjacency ships as raw fp8e4m3 counts (exact); the fp16
    fallback ships pre-scaled B^T blocks (counts > 16 only).
    """
    import concourse.mybir as mybir
    import concourse.tile as tile
    from concourse import bacc

    key = (cpc, cap, D, in_c, f_out, fp8_path)
    if key in _prog_cache:
        return _prog_cache[key]

    kc = in_c // P           # k chunks (step-1 stationary cols / step-2 contraction)
    sch = cap // P           # s chunks per cluster (step-1 contraction)
    fc = f_out // P          # f chunks (step-2 output partitions)
    f16 = mybir.dt.float16
    f32 = mybir.dt.float32
    a_dt = mybir.dt.float8e4 if fp8_path else f16

    # load-group sizes, fine-grained early so cluster data lands in
    # consumption order on the single Sync ring (sem fires per group)
    XGS = [1, 1, 1, 2, 4, 4][:cpc]
    AGS = [1, 1, 2, 3, 3, 3][:cpc]
    for sizes in (XGS, AGS):
        while sum(sizes) < cpc:
            sizes.append(min(4, cpc - sum(sizes)))
        while sum(sizes) > cpc:
            sizes[-1] -= 1
            if sizes[-1] == 0:
                sizes.pop()
    XG = max(XGS)
    AG = max(AGS)

    nc = bacc.Bacc("TRN2", target_bir_lowering=False, debug=False,
                   num_devices=N_CORES)

    XS = nc.dram_tensor("XS", [P, cpc, sch, in_c], f16, kind="ExternalInput")
    WT = nc.dram_tensor("WT", [P, kc, fc, P], f16, kind="ExternalInput")
    AT = nc.dram_tensor("AT", [cpc, P, sch, D], a_dt, kind="ExternalInput")
    YT = nc.dram_tensor("YT", [cpc, f_out, D], f16, kind="ExternalOutput")

    with tile.TileContext(nc) as tc:
        with (
            tc.tile_pool(name="w", bufs=1) as w_pool,
            tc.tile_pool(name="jk", bufs=1) as jk_pool,
            tc.tile_pool(name="xt", bufs=4) as xt_pool,
            tc.tile_pool(name="at", bufs=4) as at_pool,
            tc.tile_pool(name="tsb", bufs=4) as tsb_pool,
            tc.tile_pool(name="out", bufs=4) as out_pool,
            tc.tile_pool(name="psw", bufs=1, space="PSUM") as psw_pool,
            tc.tile_pool(name="ps1", bufs=3, space="PSUM") as ps1_pool,
            tc.tile_pool(name="ps2", bufs=4, space="PSUM") as ps2_pool,
        ):
            # warm-up burst: junk matmuls with no input deps keep the PE
            # busy through the initial DMA latency and un-throttle HAM
            junk = jk_pool.tile([P, P], f16)
            jout = jk_pool.tile([P, 8], f16)
            nc.gpsimd.memset(junk[:], 0.0)
            psw = psw_pool.tile([P, P], f32)
            for i in range(WARM_MMS):
                nc.tensor.matmul(
                    psw[:], lhsT=junk[:], rhs=junk[:],
                    start=(i == 0), stop=(i == WARM_MMS - 1),
                )
            nc.vector.tensor_copy(jout[:], psw[:, :8])

            wt = w_pool.tile([P, kc, fc, P], f16)

            def starts_of(sizes):
                of, c0 = {}, 0
                for g in sizes:
                    for c in range(c0, c0 + g):
                        of[c] = (c0, g)
                    c0 += g
                return of

            xg_of = starts_of(XGS)
            ag_of = starts_of(AGS)

            xt = at = None
            for c in range(cpc):
                a0, ag = ag_of[c]
                if c == a0:
                    at = at_pool.tile([P, AG, sch, D], a_dt)
                    if ag == 1:
                        nc.sync.dma_start(at[:, 0], AT[a0])
                    else:
                        nc.sync.dma_start(
                            at[:, :ag],
                            AT[a0:a0 + ag].rearrange("c p s d -> p c s d"),
                        )
                c0, g = xg_of[c]
                if c == c0:
                    xt = xt_pool.tile([P, XG, sch, in_c], f16)
                    nc.sync.dma_start(xt[:, :g], XS[:, c0:c0 + g])
                if c == 0:
                    # W behind cluster 0's loads; needed only by step 2
                    nc.sync.dma_start(wt[:], WT[:])
                xi = c - c0
                ci = c - a0

                # step 1: T^T = sum_s Xs-chunk^T x At rows, k on partitions
                tsb = tsb_pool.tile([P, kc, D], f16)
                for k in range(kc):
                    ps = ps1_pool.tile([P, D], f32)
                    for s in range(sch):
                        nc.tensor.matmul(
                            ps[:],
                            lhsT=xt[:, xi, s, k * P:(k + 1) * P],
                            rhs=at[:, ci, s, :],
                            start=(s == 0),
                            stop=(s == sch - 1),
                        )
                    if k == 0:
                        nc.vector.tensor_copy(tsb[:, k, :], ps[:])
                    else:
                        nc.scalar.copy(tsb[:, k, :], ps[:])

                # step 2: Y^T = sum_k W-chunk^T x T^T, f on partitions
                ot = out_pool.tile([P, fc, D], f16)
                for f in range(fc):
                    ps = ps2_pool.tile([P, D], f32)
                    for k in range(kc):
                        nc.tensor.matmul(
                            ps[:],
                            lhsT=wt[:, k, f, :],
                            rhs=tsb[:, k, :],
                            start=(k == 0),
                            stop=(k == kc - 1),
                        )
                    if f == 0:
                        nc.vector.tensor_copy(ot[:, f, :], ps[:])
                    else:
                        nc.scalar.copy(ot[:, f, :], ps[:])
                # stores ride the Scalar HWDGE ring: keeps the Sync ring
                # pure loads, and the preceding ACT on the same queue
                # means the store's wait is already satisfied when issued
                YTc = YT[c].rearrange("(f p) d -> p f d", p=P)
                if c == cpc - 1:
                    # split the final store so f0 ships while f1 drains
                    for f in range(fc):
                        nc.scalar.dma_start(YTc[:, f:f + 1], ot[:, f:f + 1])
                else:
                    nc.scalar.dma_start(YTc, ot[:])

    nc.compile()
    _prog_cache[key] = nc
    return nc


def _host_prep(X, W, b, assign, full_ei):
    """Shard + preprocess. Returns (in_maps, fp8_path, gather info)."""
    n, in_c = X.shape
    f_out = W.shape[1]
    src = full_ei[0].astype(np.int64)
    dst = full_ei[1].astype(np.int64)
    a_s = assign[src]
    intra = a_s == assign[dst]
    es, ed = src[intra], dst[intra]

    deg = np.ones(n, np.float32)
    np.add.at(deg, ed, np.float32(1))
    dis = (1.0 / np.sqrt(deg)).astype(np.float32)

    has_edge = np.zeros(N_CLUSTERS, bool)
    has_edge[np.unique(a_s[intra])] = True

    sizes = np.bincount(assign, minlength=N_CLUSTERS)
    cpc = -(-N_CLUSTERS // N_CORES)            # clusters per core
    cap = max(512, int(-(-sizes.max() // P)) * P)  # padded cluster size
    D = int(sizes.max()) if sizes.max() <= 512 else cap  # true d extent
    kc = in_c // P
    fc = f_out // P
    sch = cap // P

    starts = np.zeros(N_CLUSTERS + 1, np.int64)
    starts[1:] = np.cumsum(sizes)
    order = np.argsort(assign, kind="stable")
    pos = np.empty(n, np.int64)
    pos[order] = np.arange(n) - starts[assign[order]]

    ctot = cpc * N_CORES
    # At blocks: At[c][s, d] = #edges(s->d) + [s==d]
    At = np.zeros((ctot, cap, cap), np.uint16)
    np.add.at(At, (assign[es], pos[es], pos[ed]), 1)
    At[assign, pos, pos] += 1
    fp8_path = int(At.max()) <= 16    # integers <= 16 are exact in e4m3

    Xp = np.zeros((ctot, cap, in_c), np.float32)
    if fp8_path:
        # fold dis[s] into the node features; host applies dis[d] at the end
        Xp[assign, pos] = X * dis[:, None]
        import concourse.mybir as mybir
        At_send = At.astype(mybir.dt.np(mybir.dt.float8e4))
    else:
        # rare fallback: fully pre-scaled B^T blocks in fp16
        Xp[assign, pos] = X
        DISp = np.zeros((ctot, cap), np.float32)
        DISp[assign, pos] = dis
        At_send = (At.astype(np.float32)
                   * DISp[:, :, None] * DISp[:, None, :]).astype(np.float16)
    # [c, s, d] -> [c, p, so, d]: partition p holds rows s = so*P + p
    At_send = np.ascontiguousarray(
        At_send.reshape(-1, sch, P, cap).transpose(0, 2, 1, 3)[..., :D])
    # X: [c, s, k] -> [p, c, so, k]
    XS_all = np.ascontiguousarray(
        Xp.reshape(ctot, sch, P, in_c).transpose(2, 0, 1, 3)
    ).astype(np.float16)
    WT_send = np.ascontiguousarray(
        W.astype(np.float32).reshape(kc, P, fc, P).transpose(1, 0, 2, 3)
    ).astype(np.float16)

    in_maps = []
    for i in range(N_CORES):
        in_maps.append({
            "XS": np.ascontiguousarray(XS_all[:, i * cpc:(i + 1) * cpc]),
            "WT": WT_send,
            "AT": At_send[i * cpc:(i + 1) * cpc],
        })
    return in_maps, fp8_path, (cpc, cap, D, has_edge, pos, dis)


def _run(inputs, trace=False, tmpdir=None):
    from concourse.bass_utils import run_bass_kernel_spmd

    X = np.asarray(inputs["X"], np.float32)
    W = np.asarray(inputs["W"], np.float32)
    b = np.asarray(inputs["b"], np.float32)
    assign = np.asarray(inputs["assign"])
    full_ei = np.asarray(inputs["full_ei"])

    n, in_c = X.shape
    f_out = W.shape[1]
    in_maps, fp8_path, (cpc, cap, D, has_edge, pos, dis) = _host_prep(
        X, W, b, assign, full_ei)
    nc = _build_program(cpc, cap, D, in_c, f_out, fp8_path)

    res = run_bass_kernel_spmd(
        nc, in_maps, core_ids=list(range(N_CORES)),
        trace=trace, tmpdir=tmpdir,
    )
    # YT: [core][cpc, f_out, D]; row n lives at [core, lc, :, pos]
    YTdev = np.stack([res.results[i]["YT"] for i in range(N_CORES)])
    if YTdev.dtype != np.float32:
        YTdev = YTdev.astype(np.float32)

    c = assign.astype(np.int64)
    core = c // cpc
    lc = c % cpc
    Y = YTdev[core, lc, :, pos]
    if fp8_path:
        Y *= dis[:, None]
    Y += b[None, :].astype(np.float32)
    miss = ~has_edge[c]
    if miss.any():
        Y[miss] = X[miss]
    return Y, res


def kernel(**inputs) -> np.ndarray:
    Y, _ = _run(inputs)
    return Y
